# revision 12
# baseline (speedup 1.0000x reference)
"""Trainium2 Bass kernel for nn_EnvironmentEmbedder.

Sharding: pure data parallel. Core i processes batch slice [128*i : 128*(i+1)],
with batch elements mapped to SBUF partitions ([128, free] tiles everywhere).

Per-core compute layout (output = [128, 161*625] f32, channel-major free dim):
  ch   0..127  (static_c + dynamic_c) * obs      static+dynamic summed by
                                                 DMA-accumulate (CCE add), one
                                                 DVE tensor_tensor mul each
  ch 128       obstacle * obs
  ch 129       observability_current * obs
  ch 130       obs * obs
  ch 131..136  shuffle(prev_visitations)_j * 0.5 * obs
  ch 137       sum_k(vis_k) * obs
  ch 138       leader * obs
  ch 139       follower * obs
  ch 140..145  shuffle(all_prev_targets)_j * 0.5 * obs
  ch 146..151  shuffle(previous_target)_j * obs
  ch 152       0.5 * sum_k(atgt_k) * obs
  ch 153       sum_k(ptgt_k) * obs
  ch 154       1.0
  ch 155..160  one_hot(rot)
where obs := observability_in_memory.

The egocentric shuffle out_j = x_{(j - rot) % 6} is computed with per-partition
one-hot masks R_r = (rot == r):  out_j = sum_r R_r * x_{(j-r)%6}.  The obs
multiply is folded in by premultiplying the 6 source channels by obs once, and
the 0.5 scaling is folded into the masks.
"""

import sys

sys.path.insert(0, "/opt/trn_rl_repo")

from contextlib import ExitStack

import numpy as np

import concourse.bass as bass
import concourse.tile as tile
from concourse import bacc, mybir
from concourse.bass_utils import run_bass_kernel_spmd

F32 = mybir.dt.float32
I32 = mybir.dt.int32
ALU = mybir.AluOpType

B = 1024
N_CORES = 8
BS = B // N_CORES  # 128 batch elements per core = SBUF partitions
EMB = 128
HW = 625  # 25*25
NROT = 6
NCH = EMB + 33  # 161 output channels

ENV_CHUNK = 8  # env channels per streamed tile
STAGE_CHUNKS = [(128, 9), (137, 8), (145, 8), (153, 8)]  # (start_ch, n_ch)


def build_body(nc, tc, ctx, t_in, t_out):
    pool = ctx.enter_context(tc.tile_pool(name="resident", bufs=1))
    stage_pool = ctx.enter_context(tc.tile_pool(name="stage", bufs=2))
    env_s_pool = ctx.enter_context(tc.tile_pool(name="env_s", bufs=2))
    env_d_pool = ctx.enter_context(tc.tile_pool(name="env_d", bufs=2))

    # ---- resident loads ----
    def load(name, shape, dtype=F32):
        t = pool.tile(shape, dtype, tag=name)
        nc.sync.dma_start(t[:], t_in[name][:])
        return t

    rot_t = load("rotations", [BS, 1], I32)
    obs_t = load("observability_in_memory", [BS, HW])
    obst_t = load("obstacle_mask", [BS, HW])
    ocur_t = load("observability_current", [BS, HW])
    lead_t = load("leader_location", [BS, HW])
    foll_t = load("follower_location", [BS, HW])
    vis_t = load("previous_visitations", [BS, NROT * HW])
    atgt_t = load("all_previous_targets", [BS, NROT * HW])
    ptgt_t = load("previous_target", [BS, NROT * HW])

    # ---- constants: masks, replicated obs, ones ----
    R = []   # R[r]  = (rot == r)            [128, 1] f32
    Rh = []  # Rh[r] = 0.5 * (rot == r)
    for r in range(NROT):
        rt = pool.tile([BS, 1], F32, tag=f"R{r}")
        nc.vector.tensor_scalar(rt[:], rot_t[:], r, None, op0=ALU.is_equal)
        R.append(rt)
        rh = pool.tile([BS, 1], F32, tag=f"Rh{r}")
        nc.vector.tensor_scalar_mul(rh[:], rt[:], 0.5)
        Rh.append(rh)

    n_rep = max(NROT, ENV_CHUNK)
    obs_rep = pool.tile([BS, n_rep * HW], F32, tag="obs_rep")
    for k in range(n_rep):
        nc.vector.tensor_copy(obs_rep[:, k * HW:(k + 1) * HW], obs_t[:])

    ones_t = pool.tile([BS, HW], F32, tag="ones")
    nc.vector.memset(ones_t[:], 1.0)

    # ---- premultiply the 6-channel tensors by obs (in place) ----
    for xt in (vis_t, atgt_t, ptgt_t):
        nc.vector.tensor_mul(xt[:], xt[:], obs_rep[:, :NROT * HW])

    def emit_shuffle(slot, xp, masks, j):
        # slot = sum_r masks[r] * xp[:, ((j - r) % 6)]
        nc.scalar.mul(slot, xp[:, j * HW:(j + 1) * HW], masks[0][:])
        for r in range(1, NROT):
            k = (j - r) % NROT
            nc.vector.scalar_tensor_tensor(
                slot, xp[:, k * HW:(k + 1) * HW], masks[r][:], slot,
                op0=ALU.mult, op1=ALU.add)

    def emit_chsum(slot, xp):
        nc.vector.tensor_reduce(
            slot, xp[:].rearrange("p (c x) -> p x c", c=NROT),
            axis=mybir.AxisListType.X, op=ALU.add)

    def emit_channel(ch, slot):
        if ch == 128:
            nc.vector.tensor_mul(slot, obst_t[:], obs_t[:])
        elif ch == 129:
            nc.vector.tensor_mul(slot, ocur_t[:], obs_t[:])
        elif ch == 130:
            nc.vector.tensor_mul(slot, obs_t[:], obs_t[:])
        elif 131 <= ch <= 136:
            emit_shuffle(slot, vis_t, Rh, ch - 131)
        elif ch == 137:
            emit_chsum(slot, vis_t)
        elif ch == 138:
            nc.vector.tensor_mul(slot, lead_t[:], obs_t[:])
        elif ch == 139:
            nc.vector.tensor_mul(slot, foll_t[:], obs_t[:])
        elif 140 <= ch <= 145:
            emit_shuffle(slot, atgt_t, Rh, ch - 140)
        elif 146 <= ch <= 151:
            emit_shuffle(slot, ptgt_t, R, ch - 146)
        elif ch == 152:
            emit_chsum(slot, atgt_t)
            nc.vector.tensor_scalar_mul(slot, slot, 0.5)
        elif ch == 153:
            emit_chsum(slot, ptgt_t)
        elif ch == 154:
            nc.vector.memset(slot, 1.0)
        else:  # 155..160: compass one-hot
            nc.scalar.mul(slot, ones_t[:], R[ch - 155][:])

    # ---- env stream interleaved with the small channels ----
    # The env stream (123 MB of 141 MB) is DMA-bound: per 8-channel chunk the
    # DMA moves 7.7 MB (~23 us) while DVE needs only ~11 us. Engine streams
    # execute in order, so the small-channel DVE work is spread between env
    # chunks to fill that slack, and stage writes land mid-stream instead of
    # as a serial tail.
    ch_queue = []
    for ck, (start_ch, n_ch) in enumerate(STAGE_CHUNKS):
        for i in range(n_ch):
            ch_queue.append((ck, start_ch, n_ch, i))
    stage_tiles = {}

    def emit_small(budget):
        while budget > 0 and ch_queue:
            ck, start_ch, n_ch, i = ch_queue.pop(0)
            if ck not in stage_tiles:
                stage_tiles[ck] = stage_pool.tile(
                    [BS, n_ch * HW], F32, tag="stage", name=f"stage{ck}")
            emit_channel(start_ch + i, stage_tiles[ck][:, i * HW:(i + 1) * HW])
            if i == n_ch - 1:
                nc.scalar.dma_start(
                    t_out[:, start_ch * HW:(start_ch + n_ch) * HW],
                    stage_tiles[ck][:])
            budget -= 1

    w = ENV_CHUNK * HW
    env_total = EMB // ENV_CHUNK
    for c in range(env_total):
        cols = slice(c * w, (c + 1) * w)
        s_tile = env_s_pool.tile([BS, w], F32, tag="env_s")
        nc.sync.dma_start(s_tile[:], t_in["embedded_static"][:, cols])
        d_tile = env_d_pool.tile([BS, w], F32, tag="env_d")
        nc.sync.dma_start(d_tile[:], t_in["embedded_dynamic"][:, cols])
        nc.vector.tensor_add(s_tile[:], s_tile[:], d_tile[:])
        nc.vector.tensor_mul(s_tile[:], s_tile[:], obs_rep[:, :w])
        nc.scalar.dma_start(t_out[:, cols], s_tile[:])
        if c >= 2:
            emit_small(3)
    emit_small(len(ch_queue))

def build_nc():
    nc = bacc.Bacc("TRN2", target_bir_lowering=False, debug=False)
    t_in = {
        "embedded_static": nc.dram_tensor(
            "embedded_static", [BS, EMB * HW], F32, kind="ExternalInput"),
        "embedded_dynamic": nc.dram_tensor(
            "embedded_dynamic", [BS, EMB * HW], F32, kind="ExternalInput"),
        "obstacle_mask": nc.dram_tensor(
            "obstacle_mask", [BS, HW], F32, kind="ExternalInput"),
        "observability_current": nc.dram_tensor(
            "observability_current", [BS, HW], F32, kind="ExternalInput"),
        "observability_in_memory": nc.dram_tensor(
            "observability_in_memory", [BS, HW], F32, kind="ExternalInput"),
        "previous_visitations": nc.dram_tensor(
            "previous_visitations", [BS, NROT * HW], F32, kind="ExternalInput"),
        "all_previous_targets": nc.dram_tensor(
            "all_previous_targets", [BS, NROT * HW], F32, kind="ExternalInput"),
        "previous_target": nc.dram_tensor(
            "previous_target", [BS, NROT * HW], F32, kind="ExternalInput"),
        "leader_location": nc.dram_tensor(
            "leader_location", [BS, HW], F32, kind="ExternalInput"),
        "follower_location": nc.dram_tensor(
            "follower_location", [BS, HW], F32, kind="ExternalInput"),
        "rotations": nc.dram_tensor(
            "rotations", [BS, 1], I32, kind="ExternalInput"),
    }
    t_out = nc.dram_tensor("out", [BS, NCH * HW], F32, kind="ExternalOutput")
    with tile.TileContext(nc) as tc, ExitStack() as ctx:
        build_body(nc, tc, ctx, t_in, t_out)
    nc.compile()
    return nc


def make_in_maps(inputs):
    arrs = {k: np.ascontiguousarray(np.asarray(v)) for k, v in inputs.items()}
    flat = {
        "embedded_static": arrs["embedded_static"].reshape(B, EMB * HW),
        "embedded_dynamic": arrs["embedded_dynamic"].reshape(B, EMB * HW),
        "obstacle_mask": arrs["obstacle_mask"].reshape(B, HW),
        "observability_current": arrs["observability_current"].reshape(B, HW),
        "observability_in_memory": arrs["observability_in_memory"].reshape(B, HW),
        "previous_visitations": arrs["previous_visitations"].reshape(B, NROT * HW),
        "all_previous_targets": arrs["all_previous_targets"].reshape(B, NROT * HW),
        "previous_target": arrs["previous_target"].reshape(B, NROT * HW),
        "leader_location": arrs["leader_location"].reshape(B, HW),
        "follower_location": arrs["follower_location"].reshape(B, HW),
        "rotations": arrs["rotations"].reshape(B, 1).astype(np.int32),
    }
    return [
        {k: v[i * BS:(i + 1) * BS] for k, v in flat.items()}
        for i in range(N_CORES)
    ]


def kernel(**inputs) -> np.ndarray:
    nc = build_nc()
    in_maps = make_in_maps(inputs)
    res = run_bass_kernel_spmd(nc, in_maps, list(range(N_CORES)))
    return np.concatenate(
        [r["out"].reshape(BS, NCH, 25, 25) for r in res.results], axis=0)


if __name__ == "__main__":
    rng = np.random.default_rng(0)
    demo = {
        "embedded_static": rng.standard_normal((B, EMB, 25, 25), np.float32),
        "embedded_dynamic": rng.standard_normal((B, EMB, 25, 25), np.float32),
        "obstacle_mask": rng.random((B, 25, 25), dtype=np.float32),
        "observability_current": rng.random((B, 25, 25), dtype=np.float32),
        "observability_in_memory": rng.random((B, 25, 25), dtype=np.float32),
        "previous_visitations": rng.random((B, NROT, 25, 25), dtype=np.float32),
        "all_previous_targets": rng.random((B, NROT, 25, 25), dtype=np.float32),
        "previous_target": rng.random((B, NROT, 25, 25), dtype=np.float32),
        "leader_location": rng.random((B, 25, 25), dtype=np.float32),
        "follower_location": rng.random((B, 25, 25), dtype=np.float32),
        "rotations": rng.integers(0, NROT, (B,), dtype=np.int32),
    }
    out = kernel(**demo)
    print("out", out.shape, out.dtype)


# revision 18
# speedup vs baseline: 1.0340x; 1.0340x over previous
"""Trainium2 Bass kernel for nn_EnvironmentEmbedder.

Sharding: pure data parallel. Core i processes batch slice [128*i : 128*(i+1)],
with batch elements mapped to SBUF partitions ([128, free] tiles everywhere).

Per-core compute layout (output = [128, 161*625] f32, channel-major free dim):
  ch   0..127  (static_c + dynamic_c) * obs      streamed in 8-channel chunks
                                                 (20 KB DMA rows), DVE add+mul
                                                 in place
  ch 128       obstacle * obs
  ch 129       observability_current * obs
  ch 130       obs * obs
  ch 131..136  shuffle(prev_visitations)_j * 0.5 * obs
  ch 137       sum_k(vis_k) * obs
  ch 138       leader * obs
  ch 139       follower * obs
  ch 140..145  shuffle(all_prev_targets)_j * 0.5 * obs
  ch 146..151  shuffle(previous_target)_j * obs
  ch 152       0.5 * sum_k(atgt_k) * obs
  ch 153       sum_k(ptgt_k) * obs
  ch 154       1.0
  ch 155..160  one_hot(rot)
where obs := observability_in_memory.

The egocentric shuffle out_j = x_{(j - rot) % 6} is computed with per-partition
one-hot masks R_r = (rot == r):  out_j = sum_r R_r * x_{(j-r)%6}.  The obs
multiply is folded in by premultiplying the 6 source channels by obs once, and
the 0.5 scaling is folded into the masks.
"""

import sys

sys.path.insert(0, "/opt/trn_rl_repo")

from contextlib import ExitStack

import numpy as np

import concourse.bass as bass
import concourse.tile as tile
from concourse import bacc, mybir
from concourse.bass_utils import run_bass_kernel_spmd

F32 = mybir.dt.float32
I32 = mybir.dt.int32
ALU = mybir.AluOpType

B = 1024
N_CORES = 8
BS = B // N_CORES  # 128 batch elements per core = SBUF partitions
EMB = 128
HW = 625  # 25*25
NROT = 6
NCH = EMB + 33  # 161 output channels

ENV_CHUNK = 8  # env channels per streamed tile
PACK_LAYOUT = [("obs", HW), ("obstacle", HW), ("ocur", HW), ("leader", HW),
               ("follower", HW), ("vis", NROT * HW), ("atgt", NROT * HW),
               ("ptgt", NROT * HW), ("rot", 1)]
PACK_W = sum(w for _, w in PACK_LAYOUT)  # 14376 floats per partition
STAGE_CHUNKS = [(128, 9), (137, 8), (145, 8), (153, 8)]  # (start_ch, n_ch)


def build_body(nc, tc, ctx, t_in, t_out):
    pool = ctx.enter_context(tc.tile_pool(name="resident", bufs=1))
    stage_pool = ctx.enter_context(tc.tile_pool(name="stage", bufs=2))
    env_s_pool = ctx.enter_context(tc.tile_pool(name="env_s", bufs=2))
    env_d_pool = ctx.enter_context(tc.tile_pool(name="env_d", bufs=2))

    # ---- resident load: all small tensors host-packed into one DMA ----
    pack_t = pool.tile([BS, PACK_W], F32, tag="pack")
    nc.sync.dma_start(pack_t[:], t_in["small_pack"][:])
    cols = {}
    off = 0
    for name, wdt in PACK_LAYOUT:
        cols[name] = pack_t[:, off:off + wdt]
        off += wdt
    obs_t = cols["obs"]
    obst_t = cols["obstacle"]
    ocur_t = cols["ocur"]
    lead_t = cols["leader"]
    foll_t = cols["follower"]
    vis_t = cols["vis"]
    atgt_t = cols["atgt"]
    ptgt_t = cols["ptgt"]
    rot_t = cols["rot"].bitcast(I32)

    # ---- constants: masks, replicated obs, ones ----
    R = []   # R[r]  = (rot == r)            [128, 1] f32
    Rh = []  # Rh[r] = 0.5 * (rot == r)
    for r in range(NROT):
        rt = pool.tile([BS, 1], F32, tag=f"R{r}")
        nc.vector.tensor_scalar(rt[:], rot_t, r, None, op0=ALU.is_equal)
        R.append(rt)
        rh = pool.tile([BS, 1], F32, tag=f"Rh{r}")
        nc.vector.tensor_scalar_mul(rh[:], rt[:], 0.5)
        Rh.append(rh)

    n_rep = max(NROT, ENV_CHUNK)
    obs_rep = pool.tile([BS, n_rep * HW], F32, tag="obs_rep")
    for k in range(n_rep):
        nc.vector.tensor_copy(obs_rep[:, k * HW:(k + 1) * HW], obs_t)

    ones_t = pool.tile([BS, HW], F32, tag="ones")
    nc.vector.memset(ones_t[:], 1.0)

    # ---- premultiply the 6-channel tensors by obs (in place) ----
    for xt in (vis_t, atgt_t, ptgt_t):
        nc.vector.tensor_mul(xt, xt, obs_rep[:, :NROT * HW])

    def emit_shuffle(slot, xp, masks, j):
        # slot = sum_r masks[r] * xp[:, ((j - r) % 6)]
        nc.scalar.mul(slot, xp[:, j * HW:(j + 1) * HW], masks[0][:])
        for r in range(1, NROT):
            k = (j - r) % NROT
            nc.vector.scalar_tensor_tensor(
                slot, xp[:, k * HW:(k + 1) * HW], masks[r][:], slot,
                op0=ALU.mult, op1=ALU.add)

    def emit_chsum(slot, xp):
        nc.vector.tensor_reduce(
            slot, xp.rearrange("p (c x) -> p x c", c=NROT),
            axis=mybir.AxisListType.X, op=ALU.add)

    def emit_channel(ch, slot):
        if ch == 128:
            nc.vector.tensor_mul(slot, obst_t, obs_t)
        elif ch == 129:
            nc.vector.tensor_mul(slot, ocur_t, obs_t)
        elif ch == 130:
            nc.vector.tensor_mul(slot, obs_t, obs_t)
        elif 131 <= ch <= 136:
            emit_shuffle(slot, vis_t, Rh, ch - 131)
        elif ch == 137:
            emit_chsum(slot, vis_t)
        elif ch == 138:
            nc.vector.tensor_mul(slot, lead_t, obs_t)
        elif ch == 139:
            nc.vector.tensor_mul(slot, foll_t, obs_t)
        elif 140 <= ch <= 145:
            emit_shuffle(slot, atgt_t, Rh, ch - 140)
        elif 146 <= ch <= 151:
            emit_shuffle(slot, ptgt_t, R, ch - 146)
        elif ch == 152:
            emit_chsum(slot, atgt_t)
            nc.vector.tensor_scalar_mul(slot, slot, 0.5)
        elif ch == 153:
            emit_chsum(slot, ptgt_t)
        elif ch == 154:
            nc.vector.memset(slot, 1.0)
        else:  # 155..160: compass one-hot
            nc.scalar.mul(slot, ones_t[:], R[ch - 155][:])

    # ---- env stream interleaved with the small channels ----
    # The env stream (123 MB of 141 MB) is DMA-bound: per 8-channel chunk the
    # DMA moves 7.7 MB (~23 us) while DVE needs only ~11 us. Engine streams
    # execute in order, so the small-channel DVE work is spread between env
    # chunks to fill that slack, and stage writes land mid-stream instead of
    # as a serial tail.
    ch_queue = []
    for ck, (start_ch, n_ch) in enumerate(STAGE_CHUNKS):
        for i in range(n_ch):
            ch_queue.append((ck, start_ch, n_ch, i))
    stage_tiles = {}

    def emit_small(budget):
        while budget > 0 and ch_queue:
            ck, start_ch, n_ch, i = ch_queue.pop(0)
            if ck not in stage_tiles:
                stage_tiles[ck] = stage_pool.tile(
                    [BS, n_ch * HW], F32, tag="stage", name=f"stage{ck}")
            emit_channel(start_ch + i, stage_tiles[ck][:, i * HW:(i + 1) * HW])
            if i == n_ch - 1:
                nc.sync.dma_start(
                    t_out[:, start_ch * HW:(start_ch + n_ch) * HW],
                    stage_tiles[ck][:])
            budget -= 1

    w = ENV_CHUNK * HW
    env_total = EMB // ENV_CHUNK
    for c in range(env_total):
        cols = slice(c * w, (c + 1) * w)
        s_tile = env_s_pool.tile([BS, w], F32, tag="env_s")
        nc.sync.dma_start(s_tile[:], t_in["embedded_static"][:, cols])
        d_tile = env_d_pool.tile([BS, w], F32, tag="env_d")
        nc.sync.dma_start(d_tile[:], t_in["embedded_dynamic"][:, cols])
        nc.vector.tensor_add(s_tile[:], s_tile[:], d_tile[:])
        nc.vector.tensor_mul(s_tile[:], s_tile[:], obs_rep[:, :w])
        nc.sync.dma_start(t_out[:, cols], s_tile[:])
        if c >= 2:
            emit_small(3)
    emit_small(len(ch_queue))

def build_nc():
    nc = bacc.Bacc("TRN2", target_bir_lowering=False, debug=False)
    t_in = {
        "embedded_static": nc.dram_tensor(
            "embedded_static", [BS, EMB * HW], F32, kind="ExternalInput"),
        "embedded_dynamic": nc.dram_tensor(
            "embedded_dynamic", [BS, EMB * HW], F32, kind="ExternalInput"),
        "small_pack": nc.dram_tensor(
            "small_pack", [BS, PACK_W], F32, kind="ExternalInput"),
    }
    t_out = nc.dram_tensor("out", [BS, NCH * HW], F32, kind="ExternalOutput")
    with tile.TileContext(nc) as tc, ExitStack() as ctx:
        build_body(nc, tc, ctx, t_in, t_out)
    nc.compile()
    return nc


def make_in_maps(inputs):
    arrs = {k: np.asarray(v) for k, v in inputs.items()}
    src = {
        "obs": arrs["observability_in_memory"].reshape(B, HW),
        "obstacle": arrs["obstacle_mask"].reshape(B, HW),
        "ocur": arrs["observability_current"].reshape(B, HW),
        "leader": arrs["leader_location"].reshape(B, HW),
        "follower": arrs["follower_location"].reshape(B, HW),
        "vis": arrs["previous_visitations"].reshape(B, NROT * HW),
        "atgt": arrs["all_previous_targets"].reshape(B, NROT * HW),
        "ptgt": arrs["previous_target"].reshape(B, NROT * HW),
        "rot": arrs["rotations"].reshape(B, 1).astype(np.int32).view(np.float32),
    }
    flat = {
        "embedded_static": np.ascontiguousarray(
            arrs["embedded_static"].reshape(B, EMB * HW)),
        "embedded_dynamic": np.ascontiguousarray(
            arrs["embedded_dynamic"].reshape(B, EMB * HW)),
        "small_pack": np.concatenate(
            [src[name] for name, _ in PACK_LAYOUT], axis=1),
    }
    return [
        {k: v[i * BS:(i + 1) * BS] for k, v in flat.items()}
        for i in range(N_CORES)
    ]


def kernel(**inputs) -> np.ndarray:
    nc = build_nc()
    in_maps = make_in_maps(inputs)
    res = run_bass_kernel_spmd(nc, in_maps, list(range(N_CORES)))
    return np.concatenate(
        [r["out"].reshape(BS, NCH, 25, 25) for r in res.results], axis=0)


if __name__ == "__main__":
    rng = np.random.default_rng(0)
    demo = {
        "embedded_static": rng.standard_normal((B, EMB, 25, 25), np.float32),
        "embedded_dynamic": rng.standard_normal((B, EMB, 25, 25), np.float32),
        "obstacle_mask": rng.random((B, 25, 25), dtype=np.float32),
        "observability_current": rng.random((B, 25, 25), dtype=np.float32),
        "observability_in_memory": rng.random((B, 25, 25), dtype=np.float32),
        "previous_visitations": rng.random((B, NROT, 25, 25), dtype=np.float32),
        "all_previous_targets": rng.random((B, NROT, 25, 25), dtype=np.float32),
        "previous_target": rng.random((B, NROT, 25, 25), dtype=np.float32),
        "leader_location": rng.random((B, 25, 25), dtype=np.float32),
        "follower_location": rng.random((B, 25, 25), dtype=np.float32),
        "rotations": rng.integers(0, NROT, (B,), dtype=np.int32),
    }
    out = kernel(**demo)
    print("out", out.shape, out.dtype)


# revision 23
# speedup vs baseline: 1.0559x; 1.0212x over previous
"""Trainium2 Bass kernel for nn_EnvironmentEmbedder.

Sharding: pure data parallel. Core i processes batch slice [128*i : 128*(i+1)],
with batch elements mapped to SBUF partitions ([128, free] tiles everywhere).

Per-core compute layout (output = [128, 161*625] f32, channel-major free dim):
  ch   0..127  (static_c + dynamic_c) * obs      streamed in 8-channel chunks
                                                 (20 KB DMA rows), DVE add+mul
                                                 in place
  ch 128       obstacle * obs
  ch 129       observability_current * obs
  ch 130       obs * obs
  ch 131..136  shuffle(prev_visitations)_j * 0.5 * obs
  ch 137       sum_k(vis_k) * obs
  ch 138       leader * obs
  ch 139       follower * obs
  ch 140..145  shuffle(all_prev_targets)_j * 0.5 * obs
  ch 146..151  shuffle(previous_target)_j * obs
  ch 152       0.5 * sum_k(atgt_k) * obs
  ch 153       sum_k(ptgt_k) * obs
  ch 154       1.0
  ch 155..160  one_hot(rot)
where obs := observability_in_memory.

The egocentric shuffle out_j = x_{(j - rot) % 6} is computed with per-partition
one-hot masks R_r = (rot == r):  out_j = sum_r R_r * x_{(j-r)%6}.  The obs
multiply is folded in by premultiplying the 6 source channels by obs once, and
the 0.5 scaling is folded into the masks.
"""

import sys

sys.path.insert(0, "/opt/trn_rl_repo")

from contextlib import ExitStack

import numpy as np

import concourse.bass as bass
import concourse.tile as tile
from concourse import bacc, mybir
from concourse.bass_utils import run_bass_kernel_spmd

F32 = mybir.dt.float32
I32 = mybir.dt.int32
ALU = mybir.AluOpType

B = 1024
N_CORES = 8
BS = B // N_CORES  # 128 batch elements per core = SBUF partitions
EMB = 128
HW = 625  # 25*25
NROT = 6
NCH = EMB + 33  # 161 output channels

ENV_CHUNK = 8  # env channels per streamed tile
PACK_LAYOUT = [("obs", HW), ("obstacle", HW), ("ocur", HW), ("leader", HW),
               ("follower", HW), ("vis", NROT * HW), ("atgt", NROT * HW),
               ("ptgt", NROT * HW), ("rot", 1)]
PACK_W = sum(w for _, w in PACK_LAYOUT)  # 14376 floats per partition
STAGE_CHUNKS = [(128, 6), (134, 6), (140, 6), (146, 6), (152, 6),
                (158, 3)]  # (start_ch, n_ch)


def build_body(nc, tc, ctx, t_in, t_out):
    pool = ctx.enter_context(tc.tile_pool(name="resident", bufs=1))
    stage_pool = ctx.enter_context(tc.tile_pool(name="stage", bufs=2))
    env_s_pool = ctx.enter_context(tc.tile_pool(name="env_s", bufs=3))
    env_d_pool = ctx.enter_context(tc.tile_pool(name="env_d", bufs=2))

    # ---- resident load: all small tensors host-packed into one DMA ----
    pack_t = pool.tile([BS, PACK_W], F32, tag="pack")
    nc.sync.dma_start(pack_t[:], t_in["small_pack"][:])
    cols = {}
    off = 0
    for name, wdt in PACK_LAYOUT:
        cols[name] = pack_t[:, off:off + wdt]
        off += wdt
    obs_t = cols["obs"]
    obst_t = cols["obstacle"]
    ocur_t = cols["ocur"]
    lead_t = cols["leader"]
    foll_t = cols["follower"]
    vis_t = cols["vis"]
    atgt_t = cols["atgt"]
    ptgt_t = cols["ptgt"]
    rot_t = cols["rot"].bitcast(I32)

    # ---- constants: masks, replicated obs, ones ----
    R = []   # R[r]  = (rot == r)            [128, 1] f32
    Rh = []  # Rh[r] = 0.5 * (rot == r)
    for r in range(NROT):
        rt = pool.tile([BS, 1], F32, tag=f"R{r}")
        nc.vector.tensor_scalar(rt[:], rot_t, r, None, op0=ALU.is_equal)
        R.append(rt)
        rh = pool.tile([BS, 1], F32, tag=f"Rh{r}")
        nc.vector.tensor_scalar_mul(rh[:], rt[:], 0.5)
        Rh.append(rh)

    obs_rep = pool.tile([BS, NROT * HW], F32, tag="obs_rep")
    for k in range(NROT):
        nc.vector.tensor_copy(obs_rep[:, k * HW:(k + 1) * HW], obs_t)

    # ---- premultiply the 6-channel tensors by obs (in place) ----
    for xt in (vis_t, atgt_t, ptgt_t):
        nc.vector.tensor_mul(xt, xt, obs_rep[:, :NROT * HW])

    def emit_shuffle(slot, xp, masks, j):
        # slot = sum_r masks[r] * xp[:, ((j - r) % 6)]
        nc.scalar.mul(slot, xp[:, j * HW:(j + 1) * HW], masks[0][:])
        for r in range(1, NROT):
            k = (j - r) % NROT
            nc.vector.scalar_tensor_tensor(
                slot, xp[:, k * HW:(k + 1) * HW], masks[r][:], slot,
                op0=ALU.mult, op1=ALU.add)

    def emit_chsum(slot, xp):
        nc.vector.tensor_reduce(
            slot, xp.rearrange("p (c x) -> p x c", c=NROT),
            axis=mybir.AxisListType.X, op=ALU.add)

    def emit_channel(ch, slot):
        if ch == 128:
            nc.vector.tensor_mul(slot, obst_t, obs_t)
        elif ch == 129:
            nc.vector.tensor_mul(slot, ocur_t, obs_t)
        elif ch == 130:
            nc.vector.tensor_mul(slot, obs_t, obs_t)
        elif 131 <= ch <= 136:
            emit_shuffle(slot, vis_t, Rh, ch - 131)
        elif ch == 137:
            emit_chsum(slot, vis_t)
        elif ch == 138:
            nc.vector.tensor_mul(slot, lead_t, obs_t)
        elif ch == 139:
            nc.vector.tensor_mul(slot, foll_t, obs_t)
        elif 140 <= ch <= 145:
            emit_shuffle(slot, atgt_t, Rh, ch - 140)
        elif 146 <= ch <= 151:
            emit_shuffle(slot, ptgt_t, R, ch - 146)
        elif ch == 152:
            emit_chsum(slot, atgt_t)
            nc.vector.tensor_scalar_mul(slot, slot, 0.5)
        elif ch == 153:
            emit_chsum(slot, ptgt_t)
        elif ch == 154:
            nc.vector.memset(slot, 1.0)
        else:  # 155..160: compass one-hot = Identity(0*obs + R[r])
            nc.scalar.activation(
                slot, obs_t, mybir.ActivationFunctionType.Identity,
                bias=R[ch - 155][:], scale=0.0)

    # ---- env stream interleaved with the small channels ----
    # The env stream (123 MB of 141 MB) is DMA-bound: per 8-channel chunk the
    # DMA moves 7.7 MB (~23 us) while DVE needs only ~11 us. Engine streams
    # execute in order, so the small-channel DVE work is spread between env
    # chunks to fill that slack, and stage writes land mid-stream instead of
    # as a serial tail.
    ch_queue = []
    for ck, (start_ch, n_ch) in enumerate(STAGE_CHUNKS):
        for i in range(n_ch):
            ch_queue.append((ck, start_ch, n_ch, i))
    stage_tiles = {}

    pending_writes = []  # deferred (out_cols, tile) DMA stores

    def emit_small(budget):
        while budget > 0 and ch_queue:
            ck, start_ch, n_ch, i = ch_queue.pop(0)
            if ck not in stage_tiles:
                stage_tiles[ck] = stage_pool.tile(
                    [BS, n_ch * HW], F32, tag="stage", name=f"stage{ck}")
            emit_channel(start_ch + i, stage_tiles[ck][:, i * HW:(i + 1) * HW])
            if i == n_ch - 1:
                pending_writes.append(
                    (slice(start_ch * HW, (start_ch + n_ch) * HW),
                     stage_tiles[ck]))
            budget -= 1

    # Stores are issued one iteration late (after the NEXT chunk's loads):
    # the SP HWDGE ring is FIFO, so a store whose wait-on-DVE is unmet would
    # head-of-line-block the following loads. By the time the delayed store
    # issues, its compute finished during the preceding ~13 us of loads.
    # env_s bufs=3 keeps slot-reuse (load WAR on store) off the critical path.
    w = ENV_CHUNK * HW
    half = w // 2
    env_total = EMB // ENV_CHUNK
    for c in range(env_total):
        cols = slice(c * w, (c + 1) * w)
        s_tile = env_s_pool.tile([BS, w], F32, tag="env_s")
        nc.sync.dma_start(s_tile[:], t_in["embedded_static"][:, cols])
        d_tile = env_d_pool.tile([BS, w], F32, tag="env_d")
        nc.sync.dma_start(d_tile[:], t_in["embedded_dynamic"][:, cols])
        for out_cols, tile_ in pending_writes:
            nc.sync.dma_start(t_out[:, out_cols], tile_[:])
        pending_writes.clear()
        nc.vector.tensor_add(s_tile[:], s_tile[:], d_tile[:])
        nc.vector.tensor_mul(s_tile[:, :half], s_tile[:, :half],
                             obs_rep[:, :half])
        nc.vector.tensor_mul(s_tile[:, half:], s_tile[:, half:],
                             obs_rep[:, :half])
        pending_writes.append((cols, s_tile))
        if c >= 2:
            emit_small(3)
    emit_small(len(ch_queue))
    for out_cols, tile_ in pending_writes:
        nc.sync.dma_start(t_out[:, out_cols], tile_[:])
    pending_writes.clear()

def build_nc():
    nc = bacc.Bacc("TRN2", target_bir_lowering=False, debug=False)
    t_in = {
        "embedded_static": nc.dram_tensor(
            "embedded_static", [BS, EMB * HW], F32, kind="ExternalInput"),
        "embedded_dynamic": nc.dram_tensor(
            "embedded_dynamic", [BS, EMB * HW], F32, kind="ExternalInput"),
        "small_pack": nc.dram_tensor(
            "small_pack", [BS, PACK_W], F32, kind="ExternalInput"),
    }
    t_out = nc.dram_tensor("out", [BS, NCH * HW], F32, kind="ExternalOutput")
    with tile.TileContext(nc) as tc, ExitStack() as ctx:
        build_body(nc, tc, ctx, t_in, t_out)
    nc.compile()
    return nc


def make_in_maps(inputs):
    arrs = {k: np.asarray(v) for k, v in inputs.items()}
    src = {
        "obs": arrs["observability_in_memory"].reshape(B, HW),
        "obstacle": arrs["obstacle_mask"].reshape(B, HW),
        "ocur": arrs["observability_current"].reshape(B, HW),
        "leader": arrs["leader_location"].reshape(B, HW),
        "follower": arrs["follower_location"].reshape(B, HW),
        "vis": arrs["previous_visitations"].reshape(B, NROT * HW),
        "atgt": arrs["all_previous_targets"].reshape(B, NROT * HW),
        "ptgt": arrs["previous_target"].reshape(B, NROT * HW),
        "rot": arrs["rotations"].reshape(B, 1).astype(np.int32).view(np.float32),
    }
    flat = {
        "embedded_static": np.ascontiguousarray(
            arrs["embedded_static"].reshape(B, EMB * HW)),
        "embedded_dynamic": np.ascontiguousarray(
            arrs["embedded_dynamic"].reshape(B, EMB * HW)),
        "small_pack": np.concatenate(
            [src[name] for name, _ in PACK_LAYOUT], axis=1),
    }
    return [
        {k: v[i * BS:(i + 1) * BS] for k, v in flat.items()}
        for i in range(N_CORES)
    ]


def kernel(**inputs) -> np.ndarray:
    nc = build_nc()
    in_maps = make_in_maps(inputs)
    res = run_bass_kernel_spmd(nc, in_maps, list(range(N_CORES)))
    return np.concatenate(
        [r["out"].reshape(BS, NCH, 25, 25) for r in res.results], axis=0)


if __name__ == "__main__":
    rng = np.random.default_rng(0)
    demo = {
        "embedded_static": rng.standard_normal((B, EMB, 25, 25), np.float32),
        "embedded_dynamic": rng.standard_normal((B, EMB, 25, 25), np.float32),
        "obstacle_mask": rng.random((B, 25, 25), dtype=np.float32),
        "observability_current": rng.random((B, 25, 25), dtype=np.float32),
        "observability_in_memory": rng.random((B, 25, 25), dtype=np.float32),
        "previous_visitations": rng.random((B, NROT, 25, 25), dtype=np.float32),
        "all_previous_targets": rng.random((B, NROT, 25, 25), dtype=np.float32),
        "previous_target": rng.random((B, NROT, 25, 25), dtype=np.float32),
        "leader_location": rng.random((B, 25, 25), dtype=np.float32),
        "follower_location": rng.random((B, 25, 25), dtype=np.float32),
        "rotations": rng.integers(0, NROT, (B,), dtype=np.int32),
    }
    out = kernel(**demo)
    print("out", out.shape, out.dtype)
